# revision 1
# baseline (speedup 1.0000x reference)
"""Trainium2 Bass kernel v2 for nn_AttentionBlock (4x256x64x64 self-attention).

Sharding: 8 cores = 4 batches x 2 query-halves. Per core (batch b, half h):
  k   = fold_bn(Wk) @ x[b] + bk'       [64, 4096]  keys, bf16
  Q   = k[:, 0:2048]                   queries are a SLICE of keys (shared Wk;
                                       host rotates x columns so this core's
                                       queries sit in key columns [0, 2048))
  E   = k^T Q                          [4096, 2048], contraction 64, bf16
  P   = exp(E - S)                     S = 68 constant shift. Every row max of
                                       E is >= E_ii = ||k_i||^2 >= 0 and global
                                       max E ~ 127, so P spans [e^-68, e^59]:
                                       inside bf16/f32 range, no per-row max.
  vT  = x[b]^T @ Wv^T                  [4096, 256] bf16, + 2 ones cols -> denom
  num = P^T @ [vT | 1]                 [2048, 258] f32, DMA'd out raw
Host divides num[:, :256] by num[:, 256] (denominator), adds bv, reshapes.

exp optionally splits between the Act engine (native Exp, bias = -S) and the
DVE (Schraudolph bit-trick: u16 = rint(E*128/ln2 + (16256 - 5.5 - S*128/ln2)),
bitcast u16 -> bf16; DVE f32->u16 conversion rounds-to-nearest, verified on
hw). End-to-end rel err is 3.1e-3 either way (softmax weight errors cancel
between numerator and denominator).
"""

import numpy as np

import concourse.bass as bass
import concourse.bacc as bacc
import concourse.tile as tile
import concourse.mybir as mybir
from concourse.bass_utils import run_bass_kernel_spmd

B, C, HH, WW = 4, 256, 64, 64
HW = HH * WW          # 4096
CK, CV = 64, 256
P = 128
QH = HW // 2          # 2048 queries per core
NCORES = 8
BN_EPS = 1e-5

NJ = HW // P          # 32 key chunks
IBS = 512             # i-block size
NIB = QH // IBS       # 4 i-blocks
NQ = IBS // P         # 4 i-chunks of 128 per i-block
KC = 512              # hw chunk for the k/v projection matmuls
LAG = 4               # software pipeline depth (E/exp ahead of PV);
                      # exp fires on odd t (pair fusion), so PV(t) needs t+1

SHIFT = 68.0          # constant softmax shift: P = exp(E - SHIFT)
SCH_A = 128.0 / float(np.log(2.0))
SCH_B = 127.0 * 128.0 - 5.5 - SHIFT * SCH_A
N_DVE_EXP = 0         # DVE exp offload triggers power throttling; keep 0

F32 = mybir.dt.float32
BF16 = mybir.dt.bfloat16
U16 = mybir.dt.uint16
EXP = mybir.ActivationFunctionType.Exp
MUL = mybir.AluOpType.mult
ADD = mybir.AluOpType.add


def _emit(tc, xb, wkT, bk, wvT, out):
    from contextlib import ExitStack

    nc = tc.nc
    with ExitStack() as ctx:
        consts = ctx.enter_context(tc.tile_pool(name="consts", bufs=1))
        big = ctx.enter_context(tc.tile_pool(name="big", bufs=1))
        work = ctx.enter_context(tc.tile_pool(name="work", bufs=6))
        outp = ctx.enter_context(tc.tile_pool(name="outp", bufs=4))

        # ---- constants -------------------------------------------------
        wk_sb = consts.tile([P, 2, CK], BF16)
        nc.sync.dma_start(wk_sb, wkT.rearrange("(o p) c -> p o c", p=P))
        wv_sb = consts.tile([P, 2, CV], BF16)
        bk_sb = consts.tile([CK, 1], F32)
        ebias = consts.tile([P, 1], F32)
        nc.vector.memset(ebias, -SHIFT)
        ones_f32 = consts.tile([P, 64], F32)
        nc.gpsimd.memset(ones_f32, 1.0)

        # ---- big persistent SBUF tensors -------------------------------
        xb_sb = big.tile([P, 2, HW], BF16)
        kj = big.tile([CK, HW], BF16)
        vt = big.tile([P, NJ, CV + 2], BF16)   # vT tiles; cols 256,257 = ones
        nc.vector.tensor_copy(vt[:, :, CV:CV + 2],
                              ones_f32.rearrange("p (a b) -> p a b", b=2)[:, 0:NJ, :])

        xbr = xb.rearrange("(o p) f -> p o f", p=P)

        # ---- DMA in (xb chunk 0 issued right after wk so k-proj starts asap)
        NXB = 8
        bs = HW // NXB
        nc.sync.dma_start(xb_sb[:, :, 0:bs], xbr[:, :, 0:bs])
        nc.sync.dma_start(wv_sb, wvT.rearrange("(o p) c -> p o c", p=P))
        nc.sync.dma_start(bk_sb, bk)
        for t in range(1, NXB):
            eng = nc.sync if t % 2 == 0 else nc.gpsimd
            eng.dma_start(xb_sb[:, :, t * bs:(t + 1) * bs],
                          xbr[:, :, t * bs:(t + 1) * bs])

        pool_e = ctx.enter_context(tc.tile_pool(name="pool_e", bufs=2, space="PSUM"))
        ptp = ctx.enter_context(tc.tile_pool(name="ptp", bufs=16))

        pt_tiles = {}
        ops = {}
        pend = {}    # even t -> its eps pair tile awaiting the odd partner
        pool_o = [None]  # opened after the prologue psum pools close

        def emit_e_exp(t):
            # E matmuls write [128, 512] halves of a [128, 1024] psum pair;
            # one fused ACTIVATE per pair halves the Act fixed overhead.
            ib, jc = divmod(t, NJ)
            t0 = t - (t % 2)
            if t % 2 == 0:
                pend[t0] = pool_e.tile([P, 2 * IBS], F32, tag="e",
                                       name=f"eps_{ib}_{jc}")
            eps_pair = pend[t0]
            half = (t % 2) * IBS
            nc.tensor.matmul(eps_pair[:, half:half + IBS],
                             lhsT=kj[:, jc * P:(jc + 1) * P],
                             rhs=kj[:, ib * IBS:(ib + 1) * IBS],
                             start=True, stop=True)
            if t % 2 == 1:
                eps_pair = pend.pop(t0)
                pt = ptp.tile([P, 2 * IBS], BF16, tag="pt", name=f"pt_{ib}_{jc}")
                nc.scalar.activation(pt, eps_pair, EXP, bias=ebias)
                pt_tiles[t0] = pt[:, 0:IBS]
                pt_tiles[t0 + 1] = pt[:, IBS:2 * IBS]

        def emit_pv(t):
            ib, jc = divmod(t, NJ)
            if jc == 0:
                ops[ib] = [pool_o[0].tile([P, CV + 2], F32, tag="o",
                                          name=f"ops_{ib}_{q}") for q in range(NQ)]
            pt = pt_tiles.pop(t)
            for q in range(NQ):
                nc.tensor.matmul(ops[ib][q],
                                 lhsT=pt[:, q * P:(q + 1) * P],
                                 rhs=vt[:, jc, :],
                                 start=(jc == 0), stop=(jc == NJ - 1))
            if jc + 1 == NJ:
                for q in range(NQ):
                    ic = ib * NQ + q
                    ob = outp.tile([P, CV + 2], F32, tag="ob", name=f"ob_{ib}_{q}")
                    if ib == NIB - 1 and q % 2 == 1:
                        nc.scalar.copy(ob, ops[ib][q])
                    else:
                        nc.vector.tensor_copy(ob, ops[ib][q])
                    # spread DMA *issue* cost across two engines (each issue
                    # occupies its queue engine ~600ns)
                    if q % 2 == 0:
                        nc.sync.dma_start(out[ic * P:(ic + 1) * P, :], ob)
                    else:
                        nc.gpsimd.dma_start(out[ic * P:(ic + 1) * P, :], ob)

        # ---- prologue: k/v projections interleaved with ib=0's E+exp ---
        # 3 E tiles per chunk matches the Act engine's exp rate, so pool_e
        # never backs up into the projection matmuls.
        E_PRO = 24
        e_next = 0
        with tc.tile_pool(name="pool_pre", bufs=2, space="PSUM") as pool_pre, \
                tc.tile_pool(name="pool_v", bufs=2, space="PSUM") as pool_v:
            for t in range(HW // KC):
                sl = slice(t * KC, (t + 1) * KC)
                ps = pool_pre.tile([CK, KC], F32, tag="kps", name=f"kps_{t}")
                for o in range(2):
                    nc.tensor.matmul(ps, lhsT=wk_sb[:, o, :],
                                     rhs=xb_sb[:, o, sl],
                                     start=(o == 0), stop=(o == 1))
                nc.vector.tensor_scalar_add(kj[:, sl], ps, bk_sb)
                for jc in range(t * 4, t * 4 + 4):
                    vps = pool_v.tile([P, CV], F32, tag="v", name=f"vps_{jc}")
                    for o in range(2):
                        nc.tensor.matmul(vps,
                                         lhsT=xb_sb[:, o, jc * P:(jc + 1) * P],
                                         rhs=wv_sb[:, o, :],
                                         start=(o == 0), stop=(o == 1))
                    nc.vector.tensor_copy(vt[:, jc, 0:CV], vps)
                # E(0, jc) needs kj chunk jc//4 (lhsT) and chunk 0 (rhs)
                lim = min(E_PRO, 3 * (t + 1), 4 * (t + 1))
                while e_next < lim:
                    emit_e_exp(e_next)
                    e_next += 1

        pool_o[0] = ctx.enter_context(
            tc.tile_pool(name="pool_o", bufs=4, space="PSUM"))

        # ---- main loop: remaining E (pairs back-to-back, so the fused exp
        # starts early in each 2-tick window) + evenly paced PV drain ----
        total = NIB * NJ
        n_rest = total - E_PRO
        pv_next = 0
        for t0 in range(E_PRO, total, 2):
            emit_e_exp(t0)
            emit_e_exp(t0 + 1)
            target = (t0 + 2 - E_PRO) * total // n_rest
            while pv_next < min(target, t0 + 2 - LAG):
                emit_pv(pv_next)
                pv_next += 1
        while pv_next < total:
            emit_pv(pv_next)
            pv_next += 1


def build_nc():
    nc = bacc.Bacc(trn_type="TRN2")
    xb_d = nc.dram_tensor("xb", [C, HW], BF16, kind="ExternalInput")
    wk_d = nc.dram_tensor("wkT", [C, CK], BF16, kind="ExternalInput")
    bk_d = nc.dram_tensor("bk", [CK, 1], F32, kind="ExternalInput")
    wv_d = nc.dram_tensor("wvT", [C, CV], BF16, kind="ExternalInput")
    out_d = nc.dram_tensor("out", [QH, CV + 2], F32, kind="ExternalOutput")
    args = (xb_d[:], wk_d[:], bk_d[:], wv_d[:], out_d[:])
    with tile.TileContext(nc) as tc:
        _emit(tc, *args)
    nc.finalize()
    return nc


_NC = None


def get_nc():
    global _NC
    if _NC is None:
        _NC = build_nc()
    return _NC


def build_in_maps(inputs):
    x = np.ascontiguousarray(np.asarray(inputs["x"], np.float32))
    Wk = np.asarray(inputs["Wk"], np.float32)
    bk = np.asarray(inputs["bk"], np.float32)
    gamma = np.asarray(inputs["bn_gamma"], np.float32)
    beta = np.asarray(inputs["bn_beta"], np.float32)
    mean = np.asarray(inputs["bn_mean"], np.float32)
    var = np.asarray(inputs["bn_var"], np.float32)
    Wv = np.asarray(inputs["Wv"], np.float32)

    inv = gamma / np.sqrt(var + BN_EPS)
    wk_eff = (inv[:, None] * Wk).astype(np.float32)
    bk_eff = (inv * bk + (beta - mean * inv)).astype(np.float32)

    import ml_dtypes
    bf = ml_dtypes.bfloat16
    wkT = np.ascontiguousarray(wk_eff.T.astype(bf))     # [C, CK]
    wvT = np.ascontiguousarray(Wv.T.astype(bf))         # [C, CV]
    bk2 = np.ascontiguousarray(bk_eff.reshape(CK, 1))

    in_maps = []
    for core in range(NCORES):
        b, h = divmod(core, 2)
        xf = x[b].reshape(C, HW)
        if h == 1:
            # rotate so this core's queries sit in key columns [0, QH)
            xf = np.concatenate([xf[:, QH:], xf[:, :QH]], axis=1)
        xbc = np.ascontiguousarray(xf.astype(bf))
        in_maps.append({"xb": xbc, "wkT": wkT, "bk": bk2, "wvT": wvT})
    return in_maps


def kernel(**inputs):
    bv = np.asarray(inputs["bv"], np.float32)
    in_maps = build_in_maps(inputs)
    nc = get_nc()
    res = run_bass_kernel_spmd(nc, in_maps, core_ids=list(range(NCORES)))
    out = np.empty((B, CV, HW), np.float32)
    for core in range(NCORES):
        b, h = divmod(core, 2)
        raw = res.results[core]["out"]          # [QH, CV+2]
        o = raw[:, 0:CV] / raw[:, CV:CV + 1]
        out[b, :, h * QH:(h + 1) * QH] = o.T
    out += bv[None, :, None]
    return np.ascontiguousarray(out.reshape(B, CV, HH, WW))

